# revision 12
# baseline (speedup 1.0000x reference)
"""Trainium2 Bass kernel for nn_KKLayer (spectral channel-mix layer).

Math identity: the reference computes
    y = Re(IFFT2((A + iB) . conj(FFT2(x))))            (channel mix in freq domain)
Since channel mixing commutes with the spatial FFT and, for real x,
IFFT2(conj(FFT2(x))) is x spatially "negated" (h -> (-h) mod H, w -> (-w) mod W),
the whole layer collapses to
    y[b,o,h,w] = sum_i A[o,i] * x[b,i,(H-h)%H,(W-w)%W]
(betas drop out of the real part entirely).

The (h,w) flip is folded into the host-side shard step (a fancy-index while
casting x to bf16), so the device kernel is a pure streaming channel-matmul:

  per core (data-parallel over batch, 8 batches -> 8 cores):
    - load alphas^T (stationary weights, bf16) + pre-flipped x[b] (bf16)
    - per 2048-col chunk: 4 bf16 matmuls [K=128,M=128,N=512] -> PSUM fp32,
      one [128,2048] PSUM->SBUF copy (fp32 -> bf16, alternating DVE/ACT),
      one contiguous 512KB bf16 DMA out
    - host upcasts bf16 -> fp32

bf16 I/O halves HBM traffic (8.4MB/core, ~23.5us at the 358GB/s/core limit)
and runs the PE at 1 cycle/row instead of fp32's 4 (rel err ~4e-3 << 2e-2).
"""

import ml_dtypes
import numpy as np

import concourse.bass as bass
import concourse.bacc as bacc
import concourse.mybir as mybir
from concourse import tile
from concourse.bass_utils import run_bass_kernel_spmd

B, CIN, COUT, H, W = 8, 128, 128, 128, 128
HW = H * W          # 16384
BLK = 512           # matmul free dim (one PSUM bank of fp32)
# input DMA sizes: few big transfers keep the SDMA engines at line rate
# (~425GB/s); small transfers are latency-dominated and waste bus time
IN_COLS = [2048, 4096, 4096, 4096, 2048]
# compute/copy pipeline granularity (cols): one PSUM allocation (2 banks)
CHUNK = 1024
# output DMA granularity (512KB)
OUT_COLS = 2048
N_WARMUP = 8        # junk matmuls to ramp the PE p-state before data lands
N_CORES = 8

F32 = mybir.dt.float32
BF16 = mybir.dt.bfloat16
NP_BF16 = ml_dtypes.bfloat16

# (-h) % H index for the host-side spatial flip
_FLIP = (-np.arange(H)) % H


def _build_nc():
    nc = bacc.Bacc(None, target_bir_lowering=False)
    x = nc.dram_tensor("x", [CIN, HW], BF16, kind="ExternalInput")
    wT = nc.dram_tensor("wT", [CIN, COUT], BF16, kind="ExternalInput")
    y = nc.dram_tensor("y", [COUT, HW], BF16, kind="ExternalOutput")

    in_offs = np.cumsum([0] + IN_COLS)
    with tile.TileContext(nc) as tc:
        with (
            tc.tile_pool(name="wp", bufs=1) as wpool,
            tc.tile_pool(name="xp", bufs=1) as xpool,
            tc.tile_pool(name="yp", bufs=1) as ypool,
            tc.tile_pool(name="ps", bufs=4, space="PSUM") as pspool,
        ):
            # all input DMAs up front: the HWDGE ring drains them FIFO at
            # line rate; compute is bus-hidden so it just follows along.
            # x0 goes before w so the first matmul's input is in flight first.
            xin = []
            for k, cols in enumerate(IN_COLS):
                t = xpool.tile([CIN, cols], BF16, tag=f"x{k}", name=f"xch{k}")
                nc.sync.dma_start(t[:], x[:, in_offs[k]: in_offs[k + 1]])
                xin.append(t)
                if k == 0:
                    w_t = wpool.tile([CIN, COUT], BF16)
                    nc.sync.dma_start(w_t[:], wT[:])

            # PE p-state warm-up: junk matmuls on a memset tile keep the PE
            # continuously busy from ~6.5us so real matmuls run at full clock
            junk = wpool.tile([CIN, BLK], BF16, name="junk")
            nc.gpsimd.memset(junk[:], 0.0)
            wu = pspool.tile([COUT, CHUNK], F32, tag="ps", name="ps_wu")
            for i in range(N_WARMUP):
                nc.tensor.matmul(
                    wu[:, 0:BLK], junk[:, 0:COUT], junk[:], start=True,
                    stop=True,
                )

            yts = {}
            for c in range(HW // CHUNK):
                base = c * CHUNK
                # which input tile holds this chunk's columns
                k = int(np.searchsorted(in_offs, base, side="right")) - 1
                lo = base - in_offs[k]
                ps = pspool.tile([COUT, CHUNK], F32, tag="ps", name=f"ps{c}")
                for j in range(CHUNK // BLK):
                    nc.tensor.matmul(
                        ps[:, BLK * j: BLK * (j + 1)],
                        w_t[:],
                        xin[k][:, lo + BLK * j: lo + BLK * (j + 1)],
                        start=True,
                        stop=True,
                    )
                # dedicated y tiles (no write-after-read coupling with outs);
                # each OUT_COLS-wide y tile is filled by CHUNK-wide copies on
                # both engines in parallel
                t = c // (OUT_COLS // CHUNK)
                half = base - t * OUT_COLS
                if t not in yts:
                    yts[t] = ypool.tile(
                        [COUT, OUT_COLS], BF16, tag=f"y{t}", name=f"ych{t}"
                    )
                yt = yts[t]
                if c % 2 == 0:
                    nc.vector.tensor_copy(yt[:, half: half + CHUNK], ps[:])
                else:
                    nc.scalar.copy(yt[:, half: half + CHUNK], ps[:])
                if half + CHUNK == OUT_COLS:
                    nc.sync.dma_start(
                        y[:, t * OUT_COLS: (t + 1) * OUT_COLS], yt[:]
                    )
    nc.compile()
    return nc


_NC_CACHE = {}


def _get_nc():
    if "nc" not in _NC_CACHE:
        _NC_CACHE["nc"] = _build_nc()
    return _NC_CACHE["nc"]


def make_in_maps(x, alphas):
    """Per-core input maps: bf16, with the (h,w) flip pre-applied to x."""
    x16 = np.asarray(x, dtype=np.float32).astype(NP_BF16)
    wT = np.ascontiguousarray(
        np.asarray(alphas, dtype=np.float32).T
    ).astype(NP_BF16)
    maps = []
    for c in range(N_CORES):
        xf = x16[c][:, _FLIP][:, :, _FLIP]
        maps.append(
            {"x": np.ascontiguousarray(xf.reshape(CIN, HW)), "wT": wT}
        )
    return maps


def kernel(x, alphas, betas=None, **_unused):
    nc = _get_nc()
    in_maps = make_in_maps(x, alphas)
    res = run_bass_kernel_spmd(nc, in_maps, core_ids=list(range(N_CORES)))
    out = np.stack(
        [
            res.results[c]["y"].astype(np.float32).reshape(COUT, H, W)
            for c in range(N_CORES)
        ]
    )
    return out
